# revision 21
# baseline (speedup 1.0000x reference)
"""Trainium2 Bass kernel for nn_Discriminator (conv-highway discriminator + cosine retrieval).

Model (per reference.py):
  emb = emb_w[x]                          # [64, 128, 300]
  pred     = branch(emb, conv_w*, hw_w)   # [64, 2] log-softmax
  pred_ref = branch(emb, convr_*, hwr_w)  # only rows 0..15 are used
  values[i] = sum_j cos(pred_ref[j], pred[i]);  out = log(values / values.sum())

Sharding: 80 useful row-units (64 pred rows + 16 ref rows) are split 10 per
core: core c computes the pred branch for batch rows 8c..8c+7 and the ref
branch for rows 2c, 2c+1.  Each core returns normalized log-softmax rows
[10, 2]; the host computes the tiny cosine-sum + log normalizer (O(B) work).

On-device pipeline per core:
  - indirect-DMA gather of 10*128 embedding rows -> PE-transpose to
    channel-major embT [300, 1280] (fp32r)
  - conv-as-matmul: weights host-packed per filter-shift k into [300, 1000]
    matrices; accumulate over (k, E-chunk) into PSUM per 128-feature chunk;
    segmented reduce_max pooling straight from PSUM; fused bias+ReLU (ACT)
  - highway: batch-stationary matmul h = pooled.T @ hw_wT (streams the weight
    matrices through the PE at full rate), sigmoid/relu/mix on 10x1000
  - linear + log_softmax + row L2-normalize -> [10, 2] output
All matmuls run as float32r (full PE rate, ~1e-4 relative error).
"""

import os
import sys

for _p in ("/opt/trn_rl_repo", "/root/.axon_site/_ro/trn_rl_repo"):
    if os.path.isdir(_p) and _p not in sys.path:
        sys.path.insert(0, _p)

import ml_dtypes
import numpy as np

import concourse.bass as bass
import concourse.mybir as mybir
import concourse.tile as tile
from concourse import bacc
from concourse.bass_utils import run_bass_kernel_spmd

# ---- problem constants (hardcoded per spec) ----
B, REF, L, V, E = 64, 16, 128, 50000, 300
FS = [3, 4, 5]
NF = [300, 300, 400]
F = 1000                      # sum(NF)
NCLS = 2
N_CORES = 8
RPC = 10                      # rows per core: 8 pred + 2 ref
TOK = RPC * L                 # 1280 tokens per core
# embT is split into one tile per token-position chunk so conv matmuls only
# depend on the 4 gathers that feed their chunk: widths (cols are chunk-local)
EBW = [516, 516, 384]         # chunk j covers global cols 512j .. 512j+EBW[j]
KMAX = 5

F32R = mybir.dt.float32r
F32 = mybir.dt.float32
BF16 = mybir.dt.bfloat16
I32 = mybir.dt.int32
AX = mybir.AxisListType
AFT = mybir.ActivationFunctionType
ALU = mybir.AluOpType

# E-chunks (contraction tiling) and feature chunks
ECH = [(0, 128), (128, 128), (256, 44)]
GCH = [(g * 128, min(128, F - g * 128)) for g in range(8)]
# valid filter-shifts k contributing to feature chunk g
KSET = [range(3), range(3), range(4), range(4), range(5), range(5), range(5), range(5)]
# per-chunk segments (p0, pm, f): feature sub-ranges belonging to one conv unit
SEG = [
    [(0, 128, 3)], [(0, 128, 3)],
    [(0, 44, 3), (44, 84, 4)],
    [(0, 128, 4)],
    [(0, 88, 4), (88, 40, 5)],
    [(0, 128, 5)], [(0, 128, 5)], [(0, 104, 5)],
]
# token position chunks: (start, width, n_rows, out_col0); first two = pred, last = ref
POS = [(0, 512, 4, 0), (512, 512, 4, 4), (1024, 256, 2, 8)]

_CACHE = {}


def _build_program():
    nc = bacc.Bacc("TRN2", target_bir_lowering=False, debug=False, num_devices=N_CORES)

    d_idx = nc.dram_tensor("idx", [L, RPC], I32, kind="ExternalInput")
    d_emb = nc.dram_tensor("emb", [V, E], BF16, kind="ExternalInput")
    d_wp = nc.dram_tensor("wp", [KMAX, E, F], BF16, kind="ExternalInput")
    d_wr = nc.dram_tensor("wr", [KMAX, E, F], BF16, kind="ExternalInput")
    d_hwt = nc.dram_tensor("hwt", [2, F, F], BF16, kind="ExternalInput")
    d_cb = nc.dram_tensor("cb", [F, 2], F32, kind="ExternalInput")
    d_hwb = nc.dram_tensor("hwb", [2, F], F32, kind="ExternalInput")
    d_lint = nc.dram_tensor("lint", [F, NCLS], F32, kind="ExternalInput")
    d_linb = nc.dram_tensor("linb", [1, NCLS], F32, kind="ExternalInput")
    d_ident = nc.dram_tensor("ident", [128, 128], F32, kind="ExternalInput")
    d_identb = nc.dram_tensor("identb", [128, 128], BF16, kind="ExternalInput")
    d_ones = nc.dram_tensor("ones", [1, RPC], F32, kind="ExternalInput")
    d_onesb = nc.dram_tensor("onesb", [1, RPC], BF16, kind="ExternalInput")
    d_hwbb = nc.dram_tensor("hwbb", [1, 2 * F], BF16, kind="ExternalInput")
    d_res = nc.dram_tensor("res", [RPC, NCLS], F32, kind="ExternalOutput")

    with tile.TileContext(nc) as tc:
        _emit(nc, tc, d_idx, d_emb, d_wp, d_wr, d_hwt, d_cb, d_hwb, d_lint,
              d_linb, d_ident, d_identb, d_ones, d_onesb, d_hwbb, d_res)
    nc.finalize()
    return nc


def _emit(nc, tc, d_idx, d_emb, d_wp, d_wr, d_hwt, d_cb, d_hwb, d_lint,
          d_linb, d_ident, d_identb, d_ones, d_onesb, d_hwbb, d_res):
    STAGE = int(os.environ.get("K_STAGE", "99"))
    from contextlib import ExitStack
    ctx = ExitStack()
    singles = ctx.enter_context(tc.tile_pool(name="singles", bufs=1))
    gpool = ctx.enter_context(tc.tile_pool(name="gather", bufs=3))
    convw = ctx.enter_context(tc.tile_pool(name="convw", bufs=30))
    hwtp = ctx.enter_context(tc.tile_pool(name="hwtp", bufs=5))
    hwx = ctx.enter_context(tc.tile_pool(name="hwx", bufs=8))
    small = ctx.enter_context(tc.tile_pool(name="small", bufs=4))
    ps_tp = ctx.enter_context(tc.tile_pool(name="ps_tp", bufs=1, space="PSUM"))
    ps_cv = ctx.enter_context(tc.tile_pool(name="ps_cv", bufs=5, space="PSUM"))
    ps_hw = ctx.enter_context(tc.tile_pool(name="ps_hw", bufs=2, space="PSUM"))

    # --- small constants ---
    idx_sb = singles.tile([L, RPC], I32)
    nc.sync.dma_start(out=idx_sb[:], in_=d_idx[:])
    id_b = singles.tile([128, 128], BF16)
    nc.sync.dma_start(out=id_b[:], in_=d_identb[:])

    # --- conv weight streaming (column halves: h=0 -> cols 0:512, h=1 -> 512:1000) ---
    HW_ = [(0, 512), (512, 488)]
    wsb = {}  # (branch, k, c, h) -> tile [cw, wh] F32R
    for h, (h0, wh) in enumerate(HW_):
        ks = range(4) if h == 0 else range(5)
        for k in ks:
            for c, (c0, cw) in enumerate(ECH):
                for br, dram in (("p", d_wp), ("r", d_wr)):
                    t = convw.tile([cw, wh], BF16, tag="w", name=f"w{br}{k}{c}{h}")
                    nc.sync.dma_start(
                        out=t[:], in_=dram[k, c0:c0 + cw, h0:h0 + wh])
                    wsb[(br, k, c, h)] = t

    # small late-need constants AFTER the weight queue (their per-partition
    # 8-byte patterns are descriptor-bound and would delay the weights)
    id_f = singles.tile([128, 128], F32)
    nc.sync.dma_start(out=id_f[:], in_=d_ident[:])
    cb_sb = singles.tile([128, 8, 2], F32)   # [p, g, {pred,ref}] conv biases
    for g, (g0, mg) in enumerate(GCH):
        nc.sync.dma_start(out=cb_sb[:mg, g, :], in_=d_cb[g0:g0 + mg, :])
    lint_sb = singles.tile([128, 8, NCLS], F32R)
    for g, (g0, mg) in enumerate(GCH):
        nc.sync.dma_start(out=lint_sb[:mg, g, :], in_=d_lint[g0:g0 + mg, :].bitcast(F32R))
    linb_sb = singles.tile([1, NCLS], F32R)
    nc.sync.dma_start(out=linb_sb[:], in_=d_linb[:].bitcast(F32R))
    ones_sb = singles.tile([1, RPC], F32R)
    nc.sync.dma_start(out=ones_sb[:], in_=d_ones[:].bitcast(F32R))
    hwbb_sb = singles.tile([1, 2 * F], BF16)
    nc.sync.dma_start(out=hwbb_sb[:], in_=d_hwbb[:])
    onesb_sb = singles.tile([1, RPC], BF16)
    nc.sync.dma_start(out=onesb_sb[:], in_=d_onesb[:])

    # --- PE warm-up: ~3.5us of dummy matmuls while gathers run, so the HAM
    # clock gate reaches 2.4 GHz before real work (inputs: id_b + first
    # weight tile, both early in the DMA queue; output is scratch).
    warm_ps = ps_cv.tile([128, 512], F32, tag="cv", name="warm")
    for _ in range(8):
        nc.tensor.matmul(out=warm_ps[:, :512], lhsT=id_b[:],
                         rhs=wsb[("p", 0, 0, 0)][:128, 0:512], start=True, stop=True)

    # --- embedding gather + transpose to channel-major, per pos-chunk tile ---
    embT = [[singles.tile([128, EBW[j]], BF16, tag=f"embT{c}_{j}", name=f"embT{c}_{j}")
             for j in range(3)] for c in range(3)]
    g_ts = [gpool.tile([L, E], BF16, tag=f"emb_g{r}", name=f"emb_g{r}") for r in range(RPC)]
    for r in range(RPC):
        nc.gpsimd.indirect_dma_start(
            out=g_ts[r][:], out_offset=None,
            in_=d_emb[:],
            in_offset=bass.IndirectOffsetOnAxis(ap=idx_sb[:, r:r + 1], axis=0),
        )
    for r in range(RPC):
        j, lc = divmod(r * L, 512)   # destination chunk and chunk-local col
        for c, (c0, cw) in enumerate(ECH):
            tp = ps_tp.tile([128, 128], BF16, tag="tp")
            nc.tensor.transpose(out=tp[:cw, :L], in_=g_ts[r][:, c0:c0 + cw], identity=id_b[:])
            nc.vector.tensor_copy(out=embT[c][j][:cw, lc:lc + L], in_=tp[:cw, :L])
            if lc == 0 and j > 0:
                # first 4 cols also duplicate into the previous chunk's overlap
                nc.vector.tensor_copy(out=embT[c][j - 1][:cw, 512:516], in_=tp[:cw, :4])
    # ref-chunk shift padding (cols 256..384 of chunk 2): any valid data works
    # since those products only land in pooled-out garbage cells
    for c, (c0, cw) in enumerate(ECH):
        nc.vector.tensor_copy(out=embT[c][2][:cw, 256:384], in_=embT[c][0][:cw, 0:128])

    if STAGE <= 1:
        dbg = small.tile([RPC, NCLS], F32, tag="dbg")
        nc.vector.tensor_copy(out=dbg[:], in_=embT[0][0][:RPC, :NCLS])
        nc.sync.dma_start(out=d_res[:], in_=dbg[:])
        ctx.close()
        return

    # --- conv + pool + bias/relu ---
    pooledr = [singles.tile([128, RPC], BF16, tag=f"pool{g}", name=f"pool{g}") for g in range(8)]
    for h in range(2):
        h0 = HW_[h][0]
        for g in range(4 * h, 4 * h + 4):
            g0, mg = GCH[g]
            lo = g0 - h0
            kcs = [(k, c) for k in KSET[g] for c in range(3)]
            psu = {}
            for j, (p0_, pw, nr, oc) in enumerate(POS):
                psu[j] = ps_cv.tile([128, pw], F32, tag="cv", name=f"cv{g}_{j}")
            for i, (k, c) in enumerate(kcs):
                st, sp = (i == 0), (i == len(kcs) - 1)
                cw = ECH[c][1]
                wpt = wsb[("p", k, c, h)]
                for j in (0, 1):
                    pw = POS[j][1]
                    nc.tensor.matmul(
                        out=psu[j][:mg, :pw], lhsT=wpt[:cw, lo:lo + mg],
                        rhs=embT[c][j][:cw, k:k + pw], start=st, stop=sp)
                wrt = wsb[("r", k, c, h)]
                pw = POS[2][1]
                nc.tensor.matmul(
                    out=psu[2][:mg, :pw], lhsT=wrt[:cw, lo:lo + mg],
                    rhs=embT[c][2][:cw, k:k + pw], start=st, stop=sp)
            # pooling: max over valid positions, straight from PSUM.
            # PSUM reads must start at an aligned partition, so mixed-filter
            # chunks do a full-chunk reduce with the larger filter's (smaller)
            # count, then a single-column max fixup for the smaller filter's
            # rows (base partition 0).
            pooled = small.tile([128, RPC], F32, tag="poolraw")
            f_max = max(f for (_, _, f) in SEG[g])
            cnt = L - f_max + 1
            for j, (tp0, pw, nr, oc) in enumerate(POS):
                src = psu[j][0:mg, :].rearrange("p (r t) -> p r t", r=nr)
                nc.vector.reduce_max(
                    out=pooled[0:mg, oc:oc + nr], in_=src[:, :, 0:cnt], axis=AX.X)
                for (p0, pm, f) in SEG[g]:
                    if f == f_max:
                        continue
                    assert p0 == 0
                    for t in range(cnt, L - f + 1):
                        nc.vector.tensor_tensor(
                            out=pooled[0:pm, oc:oc + nr],
                            in0=pooled[0:pm, oc:oc + nr],
                            in1=src[0:pm, :, t], op=ALU.max)
            # bias + relu (pred cols 0..7 use conv_b, ref cols 8..9 use convr_b)
            nc.scalar.activation(out=pooledr[g][:mg, 0:8], in_=pooled[:mg, 0:8],
                                 func=AFT.Relu, bias=cb_sb[:mg, g, 0:1], scale=1.0)
            nc.scalar.activation(out=pooledr[g][:mg, 8:RPC], in_=pooled[:mg, 8:RPC],
                                 func=AFT.Relu, bias=cb_sb[:mg, g, 1:2], scale=1.0)

    if STAGE <= 2:
        dbg = small.tile([RPC, NCLS], F32, tag="dbg")
        nc.vector.tensor_copy(out=dbg[:], in_=pooledr[0][:RPC, :NCLS])
        nc.sync.dma_start(out=d_res[:], in_=dbg[:])
        ctx.close()
        return

    # --- transpose pooled features to batch-major pT [10, 1000] ---
    pT = singles.tile([RPC, F], F32)
    for g, (g0, mg) in enumerate(GCH):
        tp = ps_tp.tile([128, 128], BF16, tag="tp")
        nc.tensor.transpose(out=tp[:RPC, :mg], in_=pooledr[g][:mg, :RPC],
                            identity=id_b[:mg, :mg])
        nc.vector.tensor_copy(out=pT[:, g0:g0 + mg], in_=tp[:RPC, :mg])

    # --- highway: h = pooled.T @ hw_wT + b; out = s*relu(h) + (1-s)*p ---
    hwt_sb = {}
    for b in range(2):
        for g, (g0, mg) in enumerate(GCH):
            t = hwtp.tile([128, F], BF16, tag="hwt", name=f"hwt{b}_{g}")
            nc.scalar.dma_start(out=t[:mg, :], in_=d_hwt[b, g0:g0 + mg, :])
            hwt_sb[(b, g)] = t
    # Engine partition accesses must start at an aligned base, so both
    # branches compute all 10 rows (the unused rows are free — DVE/ACT cost is
    # free-size-bound) and the pred/ref split happens in the FREE dim after
    # the transpose back to feature-major.
    ho_b = [singles.tile([RPC, F], F32, tag=f"ho{b}", name=f"ho{b}") for b in range(2)]
    for b in range(2):
        for n0, nw in ((0, 512), (512, 488)):
            hps = ps_hw.tile([RPC, 512], F32, tag="hp")
            for g, (g0, mg) in enumerate(GCH):
                nc.tensor.matmul(
                    out=hps[:RPC, :nw], lhsT=pooledr[g][:mg, :RPC],
                    rhs=hwt_sb[(b, g)][:mg, n0:n0 + nw],
                    start=(g == 0), stop=False)
            # bias via a ones-row matmul (K=1): h += 1 . hw_b[n0:n0+nw]
            nc.tensor.matmul(out=hps[:RPC, :nw], lhsT=onesb_sb[:1, :RPC],
                             rhs=hwbb_sb[:1, b * F + n0:b * F + n0 + nw],
                             start=False, stop=True)
            s = hwx.tile([RPC, 512], F32, tag="s")
            nc.scalar.activation(out=s[:RPC, :nw], in_=hps[:RPC, :nw], func=AFT.Sigmoid)
            rl = hwx.tile([RPC, 512], F32, tag="rl")
            nc.scalar.activation(out=rl[:RPC, :nw], in_=hps[:RPC, :nw], func=AFT.Relu)
            # ho = s*(relu(h) - p) + p
            nc.vector.tensor_tensor(out=rl[:RPC, :nw], in0=rl[:RPC, :nw],
                                    in1=pT[:RPC, n0:n0 + nw], op=ALU.subtract)
            nc.vector.tensor_tensor(out=rl[:RPC, :nw], in0=s[:RPC, :nw],
                                    in1=rl[:RPC, :nw], op=ALU.mult)
            nc.vector.tensor_tensor(out=ho_b[b][:RPC, n0:n0 + nw], in0=rl[:RPC, :nw],
                                    in1=pT[:RPC, n0:n0 + nw], op=ALU.add)

    if STAGE <= 3:
        dbg = small.tile([RPC, NCLS], F32, tag="dbg")
        nc.vector.tensor_copy(out=dbg[:], in_=ho_b[0][:RPC, :NCLS])
        nc.sync.dma_start(out=d_res[:], in_=dbg[:])
        ctx.close()
        return

    # --- transpose ho back to feature-major, then linear to logits [10, 2] ---
    # cols 0..7 come from the pred branch (b=0), cols 8..9 from ref (b=1)
    hoT = [small.tile([128, RPC], F32R, tag=f"hoT{g % 2}", name=f"hoT{g}") for g in range(8)]
    lps = ps_hw.tile([RPC, 512], F32, tag="hp", name="lps")
    for g, (g0, mg) in enumerate(GCH):
        for b, (c0_, c1_) in ((0, (0, 8)), (1, (8, RPC))):
            tp2 = ps_tp.tile([128, 128], F32, tag="tp")
            nc.tensor.transpose(out=tp2[:mg, :RPC], in_=ho_b[b][:, g0:g0 + mg],
                                identity=id_f[:RPC, :RPC])
            nc.vector.tensor_copy(out=hoT[g][:mg, c0_:c1_], in_=tp2[:mg, c0_:c1_])
        if STAGE >= 5:
            nc.tensor.matmul(out=lps[:RPC, :NCLS], lhsT=hoT[g][:mg, :RPC],
                             rhs=lint_sb[:mg, g, :], start=(g == 0),
                             stop=(STAGE == 5 and g == 7))
    if STAGE >= 6:
        nc.tensor.matmul(out=lps[:RPC, :NCLS], lhsT=ones_sb[:1, :RPC],
                         rhs=linb_sb[:1, :], start=False, stop=True)
    if STAGE <= 4:
        dbg = small.tile([RPC, NCLS], F32, tag="dbg")
        nc.vector.tensor_copy(out=dbg[:], in_=hoT[7][:RPC, :NCLS].bitcast(F32))
        nc.sync.dma_start(out=d_res[:], in_=dbg[:])
        ctx.close()
        return
    if STAGE <= 6:
        dbg = small.tile([RPC, NCLS], F32, tag="dbg")
        nc.vector.tensor_copy(out=dbg[:], in_=lps[:RPC, :NCLS])
        nc.sync.dma_start(out=d_res[:], in_=dbg[:])
        ctx.close()
        return

    # --- log_softmax + L2 row normalize ---
    def _dbg_out(ap):
        dbg = small.tile([RPC, NCLS], F32, tag="dbg")
        nc.vector.tensor_copy(out=dbg[:], in_=ap)
        nc.sync.dma_start(out=d_res[:], in_=dbg[:])
        ctx.close()

    mx = small.tile([RPC, 1], F32, tag="mx")
    nc.vector.reduce_max(out=mx[:], in_=lps[:RPC, :NCLS], axis=AX.X)
    t_ = small.tile([RPC, NCLS], F32, tag="t_")
    nc.vector.tensor_scalar(out=t_[:], in0=lps[:RPC, :NCLS], scalar1=mx[:],
                            scalar2=None, op0=ALU.subtract)
    if STAGE <= 7:
        return _dbg_out(t_[:])
    e_ = small.tile([RPC, NCLS], F32, tag="e_")
    se = small.tile([RPC, 1], F32, tag="se")
    nc.scalar.activation(out=e_[:], in_=t_[:], func=AFT.Exp, accum_out=se[:])
    ls = small.tile([RPC, 1], F32, tag="ls")
    nc.scalar.activation(out=ls[:], in_=se[:], func=AFT.Ln)
    pred = small.tile([RPC, NCLS], F32, tag="pred")
    nc.vector.tensor_scalar(out=pred[:], in0=t_[:], scalar1=ls[:],
                            scalar2=None, op0=ALU.subtract)
    if STAGE <= 8:
        return _dbg_out(pred[:])
    # row L2 norm; the reference's max(norm, 1e-8) clamp is a no-op here —
    # a 2-class log-softmax row always has norm >= ln(2)/sqrt(2) ~ 0.49
    sq = small.tile([RPC, NCLS], F32, tag="sq")
    nc.vector.tensor_tensor(out=sq[:], in0=pred[:], in1=pred[:], op=ALU.mult)
    n2 = small.tile([RPC, 1], F32, tag="n2")
    nc.vector.reduce_sum(out=n2[:], in_=sq[:], axis=AX.X)
    sn = small.tile([RPC, 1], F32, tag="sn")
    nc.scalar.activation(out=sn[:], in_=n2[:], func=AFT.Sqrt)
    if STAGE <= 9:
        return _dbg_out(sn[:].to_broadcast((RPC, NCLS)))
    inv = small.tile([RPC, 1], F32, tag="inv")
    nc.vector.reciprocal(out=inv[:], in_=sn[:])
    outn = small.tile([RPC, NCLS], F32, tag="outn")
    nc.vector.tensor_scalar_mul(out=outn[:], in0=pred[:], scalar1=inv[:])
    nc.sync.dma_start(out=d_res[:], in_=outn[:])
    ctx.close()


def _pack_inputs(inputs):
    """Host-side packing: per-core index slices + shared packed weight arrays."""
    f32 = np.float32
    x = np.asarray(inputs["x"]).astype(np.int32)                  # [64, 128]
    wp = np.zeros((KMAX, E, F), f32)
    wr = np.zeros((KMAX, E, F), f32)
    offs = [0, 300, 600]
    for ui, (f, n) in enumerate(zip(FS, NF)):
        o = offs[ui]
        cw = np.asarray(inputs[f"conv_w{f}"], f32)                # [f, E, n]
        cwr = np.asarray(inputs[f"convr_w{f}"], f32)
        for k in range(f):
            wp[k, :, o:o + n] = cw[k]
            wr[k, :, o:o + n] = cwr[k]
    cb = np.stack([
        np.concatenate([np.asarray(inputs[f"conv_b{f}"], f32) for f in FS]),
        np.concatenate([np.asarray(inputs[f"convr_b{f}"], f32) for f in FS]),
    ], axis=1)                                                    # [1000, 2]
    hwt = np.stack([np.asarray(inputs["hw_w"], f32).T.copy(),
                    np.asarray(inputs["hwr_w"], f32).T.copy()]
                   ).astype(ml_dtypes.bfloat16)                   # [2, 1000, 1000]
    hwb = np.stack([np.asarray(inputs["hw_b"], f32),
                    np.asarray(inputs["hwr_b"], f32)])            # [2, 1000]
    lint = np.asarray(inputs["lin_w"], f32).T.copy()              # [1000, 2]
    linb = np.asarray(inputs["lin_b"], f32).reshape(1, NCLS)
    emb = np.ascontiguousarray(np.asarray(inputs["emb_w"], f32)).astype(ml_dtypes.bfloat16)
    ident = np.eye(128, dtype=f32)

    shared = dict(emb=emb, wp=wp.astype(ml_dtypes.bfloat16),
                  wr=wr.astype(ml_dtypes.bfloat16), hwt=hwt, cb=cb, hwb=hwb,
                  lint=lint, linb=linb, ident=ident,
                  identb=ident.astype(ml_dtypes.bfloat16),
                  ones=np.ones((1, RPC), f32),
                  onesb=np.ones((1, RPC), ml_dtypes.bfloat16),
                  hwbb=hwb.reshape(1, 2 * F).astype(ml_dtypes.bfloat16))
    in_maps = []
    for c in range(N_CORES):
        rows = list(range(8 * c, 8 * c + 8)) + [2 * c, 2 * c + 1]
        idx = np.ascontiguousarray(x[rows].T)                     # [128, 10]
        in_maps.append(dict(idx=idx, **shared))
    return in_maps


def run_cores(inputs, trace=False, **kw):
    """Compile (cached) and run on 8 cores; returns (per-core results, BassKernelResults)."""
    if "nc" not in _CACHE:
        _CACHE["nc"] = _build_program()
    nc = _CACHE["nc"]
    in_maps = _pack_inputs(inputs)
    res = run_bass_kernel_spmd(nc, in_maps, list(range(N_CORES)), trace=trace, **kw)
    return res.results, res


def kernel(**inputs) -> np.ndarray:
    results, _ = run_cores(inputs)
    pn = np.concatenate([results[c]["res"][0:8] for c in range(N_CORES)])   # [64, 2]
    rn = np.concatenate([results[c]["res"][8:RPC] for c in range(N_CORES)])  # [16, 2]
    # values[i] = sum_j cos(rn_j, pn_i) = pn_i . sum_j rn_j ; out = log(values/sum)
    s = rn.sum(axis=0)
    values = pn @ s
    return np.log(values / values.sum()).astype(np.float32)
